# revision 31
# baseline (speedup 1.0000x reference)
"""Banded soft-DTW loss (normalize=True) Trainium2 Bass kernel.

Problem: x, y [32, 512, 4] f32 -> loss [32] f32
  loss = softdtw(x,y) - 0.5*(softdtw(x,x) + softdtw(y,y)), gamma=2, band=50.

Strategy (pure data parallel over 8 cores, 4 batch elements each):
  * 12 DP problems per core (xy, xx, yy for 4 batches).
  * Host precomputes bf16 matmul operands (padded b, -|b|^2/2 row with NEG
    pads, per-chunk -|a|^2/2 bias columns) so the device does no phase-0
    element work.
  * Cost matrices: per (problem, 128-row chunk) one K=4 matmul (a.b) plus a
    K=1 matmul (-|b|^2/2 broadcast); ACT exp with per-partition bias
    (-|a|^2/2) -> E = exp(-D/2) bf16; DRAM round-trip extracts the banded
    diagonal per row directly into interleaved DP layout (E at odd slots).
  * Exp-space DP: ONE fused tensor_tensor_scan per row over 202 interleaved
    slots (2 per band cell k): even slot adds S_prev[k+1], odd slot adds
    S_prev[k] and multiplies by E[k]. The scan's 2-free-dim operand APs
    (stride -2 inner on the prev-row read) fold the neighbor adds into the
    scan, eliminating the separate per-row add op.
  * Rescale by the diagonal cell every RS rows (log accumulated), clamp at
    CAP for overflow insurance.
  * Readout: R = -gamma*(ln S* + sum ln m); loss combined on-device via a
    tiny matmul with weights [-2, +1, +1].
"""
import os
import sys
from contextlib import ExitStack

import numpy as np

for _p in ("/opt/trn_rl_repo", "/root/.axon_site/_ro/trn_rl_repo"):
    if os.path.isdir(_p) and _p not in sys.path:
        sys.path.append(_p)

import concourse.bass as bass
import concourse.bacc as bacc
import concourse.mybir as mybir
import concourse.tile as tile
from concourse.bass_utils import run_bass_kernel_spmd

try:
    from ml_dtypes import bfloat16 as np_bf16
except ImportError:  # pragma: no cover
    import jax.numpy as _jnp
    np_bf16 = _jnp.bfloat16

F32 = mybir.dt.float32
BF16 = mybir.dt.bfloat16
ALU = mybir.AluOpType
ACTF = mybir.ActivationFunctionType

N = 512            # sequence length
DIM = 4            # feature dim
BPC = 4            # batch elements per core
NPROB = 3 * BPC    # DP problems per core (xy, xx, yy)
NCORE = 8
BW = 50            # band half width
NC_ = 101          # valid band cells per row
RL = 204           # DP S-row slot pitch (202 interleaved + 2 zero pads)
NCHUNK = N // 128
WIN = 231          # matmul j-window: 128 + 104 - 1
BCOLS = N + 104    # padded B width: 50 left pad + 512 + 54 right pad = 616
GROWS = 32         # rows per gather group
NGRP = N // GROWS  # 16
RING = int(os.environ.get('KRING', '8'))  # resident E-group ring depth
RS = int(os.environ.get('KRS', '16'))
NEVT = N // RS
CAP = 1e30
NEG = -20000.0     # pad dot-product value -> exp() == 0

PAIRS = [(b, 4 + b) for b in range(BPC)] \
    + [(b, b) for b in range(BPC)] \
    + [(4 + b, 4 + b) for b in range(BPC)]


def _raw_scan(q, out_ap, in0_ap, in1_ap, initial=0.0):
    """tensor_tensor_scan with 2-free-dim operand APs (bypasses the 1-free-dim
    helper assert; the TENSOR2D ISA operand encodes 2 free dims)."""
    return q.add_instruction(
        mybir.InstTensorScalarPtr(
            name=q.bass.get_next_instruction_name(),
            is_tensor_tensor_scan=True,
            is_scalar_tensor_tensor=True,
            op0=ALU.add,
            op1=ALU.mult,
            ins=[q.lower_ap(in0_ap),
                 q.lower_ap_or_imm(float(initial)),
                 q.lower_ap(in1_ap)],
            outs=[q.lower_ap(out_ap)],
        ))


def _build_nc():
    nc = bacc.Bacc("TRN2", target_bir_lowering=False, debug=False)
    a_in = nc.dram_tensor("a_in", [DIM, 8 * N], BF16, kind="ExternalInput").ap()
    b_in = nc.dram_tensor("b_in", [DIM, 8 * BCOLS], BF16, kind="ExternalInput").ap()
    bn_in = nc.dram_tensor("bn_in", [1, 8 * BCOLS], BF16, kind="ExternalInput").ap()
    ab_in = nc.dram_tensor("ab_in", [128, NCHUNK * 8], F32, kind="ExternalInput").ap()
    si_in = nc.dram_tensor("si_in", [NPROB, RL], BF16, kind="ExternalInput").ap()
    ep_in = nc.dram_tensor("ep_in", [NPROB, GROWS * RL], BF16, kind="ExternalInput").ap()
    cmat = nc.dram_tensor("cmat", [NPROB, BPC], F32, kind="ExternalInput").ap()
    out = nc.dram_tensor("out", [BPC], F32, kind="ExternalOutput").ap()
    dbg = None
    if os.environ.get('KDUMP'):
        dbg = (nc.dram_tensor(
                   "dbg", [NPROB, GROWS * RL], BF16, kind="ExternalOutput").ap(),
               nc.dram_tensor(
                   "sdbg", [NPROB, 2 * RL], BF16, kind="ExternalOutput").ap(),
               nc.dram_tensor(
                   "mdbg", [NPROB, NEVT], F32, kind="ExternalOutput").ap(),
               nc.dram_tensor(
                   "rdbg", [NPROB, NEVT + 3], F32, kind="ExternalOutput").ap())

    with tile.TileContext(nc) as tc, ExitStack() as ctx:
        _emit(ctx, tc, a_in, b_in, bn_in, ab_in, si_in, ep_in, cmat, out, dbg)
    nc.compile()
    return nc


def _emit(ctx, tc, a_in, b_in, bn_in, ab_in, si_in, ep_in, cmat, out, dbg=None):
    nc = tc.nc

    const = ctx.enter_context(tc.tile_pool(name="const", bufs=1))
    winp = ctx.enter_context(tc.tile_pool(name="winp", bufs=6))
    ps_win = ctx.enter_context(tc.tile_pool(name="ps_win", bufs=4, space="PSUM"))
    ps_misc = ctx.enter_context(tc.tile_pool(name="ps_misc", bufs=1, space="PSUM"))
    dramp = ctx.enter_context(tc.tile_pool(name="dramp", bufs=1, space="DRAM"))
    epool = ctx.enter_context(tc.tile_pool(name="epool", bufs=1))
    spool = ctx.enter_context(tc.tile_pool(name="spool", bufs=1))

    a_t = const.tile([DIM, 8 * N], BF16)
    b_t = const.tile([DIM, 8 * BCOLS], BF16)
    bn_t = const.tile([1, 8 * BCOLS], BF16)
    ab_t = const.tile([128, NCHUNK * 8], F32)
    ones_t = const.tile([1, 128], BF16)
    cm = const.tile([NPROB, BPC], F32)
    m_buf = const.tile([NPROB, NEVT], F32)
    s0 = spool.tile([NPROB, RL], BF16)
    s1 = spool.tile([NPROB, RL], BF16)
    e_ring = [epool.tile([NPROB, GROWS * RL], BF16, name=f"er{i}")
              for i in range(RING)]

    nc.sync.dma_start(a_t[:], a_in)
    nc.sync.dma_start(b_t[:], b_in)
    nc.sync.dma_start(bn_t[:], bn_in)
    nc.scalar.dma_start(ab_t[:], ab_in)
    nc.scalar.dma_start(s0[:], si_in)
    nc.scalar.dma_start(cm[:], cmat)
    nc.vector.memset(ones_t[:], 1.0)
    nc.vector.memset(m_buf[:], 1.0)
    nc.vector.memset(s1[:], 0.0)
    # interleave pattern (evens=1.0) into each ring tile; gathers overwrite odds
    for i in range(RING):
        nc.gpsimd.dma_start(e_ring[i][:], ep_in)

    scratch = dramp.tile([NPROB, NCHUNK, 128, WIN], BF16)
    sc_handle = scratch[:].tensor

    # ---- Phase 1: matmul -> exp -> DRAM -> banded gather -> interleave -----
    wq = [nc.sync, nc.scalar]
    gq = [nc.gpsimd, nc.sync, nc.scalar]

    def phase1_chunk(c):
        for pi, (sa, sb) in enumerate(PAIRS):
            pw = ps_win.tile([128, WIN], F32, name=f"pw{pi}_{c}", tag="pw")
            nc.tensor.matmul(
                pw[:],
                a_t[:, sa * N + c * 128:sa * N + (c + 1) * 128],
                b_t[:, sb * BCOLS + c * 128:sb * BCOLS + c * 128 + WIN],
                start=True, stop=False,
            )
            nc.tensor.matmul(
                pw[:],
                ones_t[:, 0:128],
                bn_t[:, sb * BCOLS + c * 128:sb * BCOLS + c * 128 + WIN],
                start=False, stop=True,
            )
            ew = winp.tile([128, WIN], BF16, name=f"ew{pi}_{c}", tag="ew")
            nc.scalar.activation(
                ew[:], pw[:], ACTF.Exp,
                bias=ab_t[:, c * 8 + sa:c * 8 + sa + 1])
            wq[pi % 2].dma_start(scratch[pi, c], ew[:])
        for h in range(128 // GROWS):
            g = c * (128 // GROWS) + h
            src = bass.AP(
                sc_handle,
                c * 128 * WIN + h * GROWS * (WIN + 1),
                [[NCHUNK * 128 * WIN, NPROB], [WIN + 1, GROWS], [1, NC_]],
            )
            stg = winp.tile([NPROB, GROWS * NC_], BF16, name=f"stg{g}",
                            tag="stg", bufs=3)
            gq[g % 3].dma_start(
                stg[:].rearrange("p (r t) -> p r t", t=NC_), src)
            dst = e_ring[g % RING][:].rearrange(
                "p (r c u) -> p r c u", c=RL // 2, u=2)[:, :, 0:NC_, 1]
            nc.gpsimd.tensor_copy(
                dst, stg[:].rearrange("p (r t) -> p r t", t=NC_))

    # ring slack = RING//4 chunks: emit 2 chunks ahead, then one more chunk
    # after each chunk's rows are consumed (preserves WAR order on ring reuse)
    for c in range(min(2, NCHUNK)):
        phase1_chunk(c)

    if dbg is not None:
        nc.sync.dma_start(dbg[0], e_ring[int(os.environ.get('KDG', '0'))][:])

    # ---- Phase 2: fused-scan DP -------------------------------------------
    v = nc.vector
    sr = [s0, s1]
    DPROWS = int(os.environ.get('KROWS', str(N)))
    for r in range(DPROWS):
        if r > 0 and r % 128 == 0 and r // 128 + 1 < NCHUNK:
            phase1_chunk(r // 128 + 1)
        prev = sr[r % 2]
        cur = sr[(r + 1) % 2]
        pap = prev[:]
        in0 = bass.AP(pap.tensor, pap.offset + 3,
                      [list(pap.ap[0]), [2, NC_], [-2, 2]])
        o_ap = cur[:].rearrange("p (c u) -> p c u", u=2)[:, 0:NC_, :]
        in1 = e_ring[(r // GROWS) % RING][:].rearrange(
            "p (r c u) -> p r c u", c=RL // 2, u=2)[:, r % GROWS][:, 0:NC_, :]
        _raw_scan(v, o_ap, in0, in1, 0.0)
        if r % RS == RS - 1:
            ev = r // RS
            v.reciprocal(m_buf[:, ev:ev + 1], cur[:, 101:102])
            v.tensor_scalar(cur[:, 0:2 * NC_], cur[:, 0:2 * NC_],
                            m_buf[:, ev:ev + 1], CAP, ALU.mult, ALU.min)

    if dbg is not None:
        sd = dbg[1].rearrange("p (a b) -> p a b", b=RL)
        nc.scalar.dma_start(sd[:, 0, :], s0[:])
        nc.scalar.dma_start(sd[:, 1, :], s1[:])
        nc.scalar.dma_start(dbg[2], m_buf[:])

    # ---- Phase 3: readout --------------------------------------------------
    # ACT's Ln table only covers inputs in ~(2^-64, 2^64); 1/m can reach e^59
    # at RS=16. Pre-scale by 2^-32: the ln(2^-32) offset is constant per event
    # and cancels in the loss combination (cmat columns sum to zero).
    ln_m = const.tile([NPROB, NEVT], F32)
    tsum = const.tile([NPROB, 1], F32)
    tfin = const.tile([NPROB, 1], F32)
    ln_s = const.tile([NPROB, 1], F32)
    nc.vector.tensor_scalar_mul(m_buf[:], m_buf[:], float(2.0 ** -32))
    nc.scalar.activation(ln_m[:], m_buf[:], ACTF.Ln)
    nc.vector.reduce_sum(tsum[:], ln_m[:], axis=mybir.AxisListType.X)
    s_last = sr[DPROWS % 2]
    nc.scalar.activation(ln_s[:], s_last[:, 101:102], ACTF.Ln)
    nc.vector.tensor_sub(tfin[:], ln_s[:], tsum[:])
    if dbg is not None:
        rd = dbg[3]
        nc.sync.dma_start(rd[:, 0:NEVT], ln_m[:])
        nc.sync.dma_start(rd[:, NEVT:NEVT + 1], tsum[:])
        nc.sync.dma_start(rd[:, NEVT + 1:NEVT + 2], ln_s[:])
        nc.sync.dma_start(rd[:, NEVT + 2:NEVT + 3], tfin[:])
    pf = ps_misc.tile([BPC, 1], F32)
    nc.tensor.matmul(pf[:], cm[:], tfin[:])
    ob = const.tile([BPC, 1], F32)
    nc.scalar.copy(ob[:], pf[:])
    nc.sync.dma_start(out, ob[:])


_NC_CACHE = None


def _get_nc():
    global _NC_CACHE
    if _NC_CACHE is None:
        _NC_CACHE = _build_nc()
    return _NC_CACHE


def _cmat_np():
    cmv = np.zeros((NPROB, BPC), np.float32)
    for m in range(BPC):
        cmv[m, m] = -2.0       # -gamma * T_xy
        cmv[4 + m, m] = 1.0    # +gamma/2 * T_xx
        cmv[8 + m, m] = 1.0    # +gamma/2 * T_yy
    return cmv


def _host_prep(xs, ys):
    """Build per-core device operands from [BPC, N, DIM] f32 slices."""
    seqs = [xs[b] for b in range(BPC)] + [ys[b] for b in range(BPC)]  # 8 x [N, D]
    a = np.empty((DIM, 8 * N), np.float32)
    b = np.zeros((DIM, 8 * BCOLS), np.float32)
    bn = np.full((1, 8 * BCOLS), NEG, np.float32)
    ab = np.empty((128, NCHUNK * 8), np.float32)
    for s, q in enumerate(seqs):
        qt = q.T  # [D, N]
        a[:, s * N:(s + 1) * N] = qt
        b[:, s * BCOLS + BW:s * BCOLS + BW + N] = qt
        nrm = -0.5 * np.sum(q * q, axis=1)  # [N]
        bn[0, s * BCOLS + BW:s * BCOLS + BW + N] = nrm
        for c in range(NCHUNK):
            ab[:, c * 8 + s] = nrm[c * 128:(c + 1) * 128]
    si = np.zeros((NPROB, RL), np.float32)
    si[:, 101] = 1.0
    ep = np.zeros((NPROB, GROWS * RL), np.float32)
    ep[:, :].reshape(NPROB, GROWS, RL)[:, :, 0::2] = 1.0
    return {
        "a_in": a.astype(np_bf16),
        "b_in": b.astype(np_bf16),
        "bn_in": bn.astype(np_bf16),
        "ab_in": ab,
        "si_in": si.astype(np_bf16),
        "ep_in": ep.astype(np_bf16),
        "cmat": _cmat_np(),
    }


def _in_maps(x, y):
    maps = []
    for k in range(NCORE):
        xs = x[BPC * k:BPC * (k + 1)]
        ys = y[BPC * k:BPC * (k + 1)]
        maps.append(_host_prep(xs, ys))
    return maps


def kernel(x: np.ndarray, y: np.ndarray) -> np.ndarray:
    x = np.ascontiguousarray(x, np.float32)
    y = np.ascontiguousarray(y, np.float32)
    B = x.shape[0]
    assert x.shape == (B, N, DIM) and B == BPC * NCORE
    nc = _get_nc()
    res = run_bass_kernel_spmd(nc, _in_maps(x, y), list(range(NCORE)))
    outs = [np.asarray(res.results[k]["out"]).reshape(BPC) for k in range(NCORE)]
    return np.concatenate(outs).astype(np.float32)


if __name__ == "__main__":
    xx = np.random.randn(32, N, DIM).astype(np.float32)
    yy = np.random.randn(32, N, DIM).astype(np.float32)
    print(kernel(xx, yy)[:4])


# revision 32
# speedup vs baseline: 1.4028x; 1.4028x over previous
"""Banded soft-DTW loss (normalize=True) Trainium2 Bass kernel.

Problem: x, y [32, 512, 4] f32 -> loss [32] f32
  loss = softdtw(x,y) - 0.5*(softdtw(x,x) + softdtw(y,y)), gamma=2, band=50.

Strategy (pure data parallel over 8 cores, 4 batch elements each):
  * 12 DP problems per core (xy, xx, yy for 4 batches).
  * Host precomputes bf16 matmul operands (padded b, -|b|^2/2 row with NEG
    pads, per-chunk -|a|^2/2 bias columns) so the device does no phase-0
    element work.
  * Cost matrices: per (problem, 128-row chunk) one K=4 matmul (a.b) plus a
    K=1 matmul (-|b|^2/2 broadcast); ACT exp with per-partition bias
    (-|a|^2/2) -> E = exp(-D/2) bf16; DRAM round-trip extracts the banded
    diagonal per row directly into interleaved DP layout (E at odd slots).
  * Exp-space DP: ONE fused tensor_tensor_scan per row over 202 interleaved
    slots (2 per band cell k): even slot adds S_prev[k+1], odd slot adds
    S_prev[k] and multiplies by E[k]. The scan's 2-free-dim operand APs
    (stride -2 inner on the prev-row read) fold the neighbor adds into the
    scan, eliminating the separate per-row add op.
  * Rescale by the diagonal cell every RS rows (log accumulated), clamp at
    CAP for overflow insurance.
  * Readout: R = -gamma*(ln S* + sum ln m); loss combined on-device via a
    tiny matmul with weights [-2, +1, +1].
"""
import os
import sys
from contextlib import ExitStack

import numpy as np

for _p in ("/opt/trn_rl_repo", "/root/.axon_site/_ro/trn_rl_repo"):
    if os.path.isdir(_p) and _p not in sys.path:
        sys.path.append(_p)

import concourse.bass as bass
import concourse.bacc as bacc
import concourse.mybir as mybir
import concourse.tile as tile
from concourse.bass_utils import run_bass_kernel_spmd

try:
    from ml_dtypes import bfloat16 as np_bf16
except ImportError:  # pragma: no cover
    import jax.numpy as _jnp
    np_bf16 = _jnp.bfloat16

F32 = mybir.dt.float32
BF16 = mybir.dt.bfloat16
ALU = mybir.AluOpType
ACTF = mybir.ActivationFunctionType

N = 512            # sequence length
DIM = 4            # feature dim
BPC = 4            # batch elements per core
NPROB = 3 * BPC    # DP problems per core (xy, xx, yy)
NCORE = 8
BW = 50            # band half width
NC_ = 101          # valid band cells per row
RL = 204           # DP S-row slot pitch (202 interleaved + 2 zero pads)
NCHUNK = N // 128
WIN = 231          # matmul j-window: 128 + 104 - 1
BCOLS = N + 104    # padded B width: 50 left pad + 512 + 54 right pad = 616
GROWS = 32         # rows per gather group
NGRP = N // GROWS  # 16
RING = int(os.environ.get('KRING', '8'))  # resident E-group ring depth
RS = int(os.environ.get('KRS', '16'))
NEVT = N // RS
CAP = 1e30
NEG = -20000.0     # pad dot-product value -> exp() == 0

PAIRS = [(b, 4 + b) for b in range(BPC)] \
    + [(b, b) for b in range(BPC)] \
    + [(4 + b, 4 + b) for b in range(BPC)]


def _raw_scan(q, out_ap, in0_ap, in1_ap, initial=0.0):
    """tensor_tensor_scan with 2-free-dim operand APs (bypasses the 1-free-dim
    helper assert; the TENSOR2D ISA operand encodes 2 free dims)."""
    return q.add_instruction(
        mybir.InstTensorScalarPtr(
            name=q.bass.get_next_instruction_name(),
            is_tensor_tensor_scan=True,
            is_scalar_tensor_tensor=True,
            op0=ALU.add,
            op1=ALU.mult,
            ins=[q.lower_ap(in0_ap),
                 q.lower_ap_or_imm(float(initial)),
                 q.lower_ap(in1_ap)],
            outs=[q.lower_ap(out_ap)],
        ))


def _build_nc():
    nc = bacc.Bacc("TRN2", target_bir_lowering=False, debug=False)
    a_in = nc.dram_tensor("a_in", [DIM, 8 * N], BF16, kind="ExternalInput").ap()
    b_in = nc.dram_tensor("b_in", [DIM, 8 * BCOLS], BF16, kind="ExternalInput").ap()
    bn_in = nc.dram_tensor("bn_in", [1, 8 * BCOLS], BF16, kind="ExternalInput").ap()
    ab_in = nc.dram_tensor("ab_in", [128, NCHUNK * 8], F32, kind="ExternalInput").ap()
    si_in = nc.dram_tensor("si_in", [NPROB, RL], BF16, kind="ExternalInput").ap()
    ep_in = nc.dram_tensor("ep_in", [NPROB, GROWS * RL], BF16, kind="ExternalInput").ap()
    cmat = nc.dram_tensor("cmat", [NPROB, BPC], F32, kind="ExternalInput").ap()
    out = nc.dram_tensor("out", [BPC], F32, kind="ExternalOutput").ap()
    dbg = None
    if os.environ.get('KDUMP'):
        dbg = (nc.dram_tensor(
                   "dbg", [NPROB, GROWS * RL], BF16, kind="ExternalOutput").ap(),
               nc.dram_tensor(
                   "sdbg", [NPROB, 2 * RL], BF16, kind="ExternalOutput").ap(),
               nc.dram_tensor(
                   "mdbg", [NPROB, NEVT], F32, kind="ExternalOutput").ap(),
               nc.dram_tensor(
                   "rdbg", [NPROB, NEVT + 3], F32, kind="ExternalOutput").ap())

    with tile.TileContext(nc) as tc, ExitStack() as ctx:
        _emit(ctx, tc, a_in, b_in, bn_in, ab_in, si_in, ep_in, cmat, out, dbg)
    nc.compile()
    return nc


def _emit(ctx, tc, a_in, b_in, bn_in, ab_in, si_in, ep_in, cmat, out, dbg=None):
    nc = tc.nc

    const = ctx.enter_context(tc.tile_pool(name="const", bufs=1))
    winp = ctx.enter_context(tc.tile_pool(name="winp", bufs=6))
    ps_win = ctx.enter_context(tc.tile_pool(name="ps_win", bufs=4, space="PSUM"))
    ps_misc = ctx.enter_context(tc.tile_pool(name="ps_misc", bufs=1, space="PSUM"))
    dramp = ctx.enter_context(tc.tile_pool(name="dramp", bufs=1, space="DRAM"))
    epool = ctx.enter_context(tc.tile_pool(name="epool", bufs=1))
    spool = ctx.enter_context(tc.tile_pool(name="spool", bufs=1))

    a_t = const.tile([DIM, 8 * N], BF16)
    b_t = const.tile([DIM, 8 * BCOLS], BF16)
    bn_t = const.tile([1, 8 * BCOLS], BF16)
    ab_t = const.tile([128, NCHUNK * 8], F32)
    ones_t = const.tile([1, 128], BF16)
    cm = const.tile([NPROB, BPC], F32)
    m_buf = const.tile([NPROB, NEVT], F32)
    s0 = spool.tile([NPROB, RL], BF16)
    s1 = spool.tile([NPROB, RL], BF16)
    e_ring = [epool.tile([NPROB, GROWS * RL], BF16, name=f"er{i}")
              for i in range(RING)]

    nc.sync.dma_start(a_t[:], a_in)
    nc.sync.dma_start(b_t[:], b_in)
    nc.sync.dma_start(bn_t[:], bn_in)
    nc.scalar.dma_start(ab_t[:], ab_in)
    nc.scalar.dma_start(s0[:], si_in)
    nc.scalar.dma_start(cm[:], cmat)
    nc.vector.memset(ones_t[:], 1.0)
    nc.vector.memset(m_buf[:], 1.0)
    nc.vector.memset(s1[:], 0.0)
    # interleave pattern (evens=1.0) into each ring tile; gathers overwrite odds
    for i in range(RING):
        nc.gpsimd.dma_start(e_ring[i][:], ep_in)

    scratch = dramp.tile([NPROB, NCHUNK, 128, WIN], BF16)
    sc_handle = scratch[:].tensor

    # ---- Phase 1: matmul -> exp -> DRAM -> banded gather -> interleave -----
    wq = [nc.sync, nc.scalar]
    gq = [nc.gpsimd, nc.sync, nc.scalar]

    def phase1_chunk(c):
        for pi, (sa, sb) in enumerate(PAIRS):
            pw = ps_win.tile([128, WIN], F32, name=f"pw{pi}_{c}", tag="pw")
            nc.tensor.matmul(
                pw[:],
                a_t[:, sa * N + c * 128:sa * N + (c + 1) * 128],
                b_t[:, sb * BCOLS + c * 128:sb * BCOLS + c * 128 + WIN],
                start=True, stop=False,
            )
            nc.tensor.matmul(
                pw[:],
                ones_t[:, 0:128],
                bn_t[:, sb * BCOLS + c * 128:sb * BCOLS + c * 128 + WIN],
                start=False, stop=True,
            )
            ew = winp.tile([128, WIN], BF16, name=f"ew{pi}_{c}", tag="ew")
            nc.scalar.activation(
                ew[:], pw[:], ACTF.Exp,
                bias=ab_t[:, c * 8 + sa:c * 8 + sa + 1])
            wq[pi % 2].dma_start(scratch[pi, c], ew[:])
        for h in range(128 // GROWS):
            g = c * (128 // GROWS) + h
            src = bass.AP(
                sc_handle,
                c * 128 * WIN + h * GROWS * (WIN + 1),
                [[NCHUNK * 128 * WIN, NPROB], [WIN + 1, GROWS], [1, NC_]],
            )
            stg = winp.tile([NPROB, GROWS * NC_], BF16, name=f"stg{g}",
                            tag="stg", bufs=3)
            gq[g % 3].dma_start(
                stg[:].rearrange("p (r t) -> p r t", t=NC_), src)
            dst = e_ring[g % RING][:].rearrange(
                "p (r c u) -> p r c u", c=RL // 2, u=2)[:, :, 0:NC_, 1]
            nc.scalar.copy(dst, stg[:].rearrange("p (r t) -> p r t", t=NC_))

    # ring slack = RING//4 chunks: emit 2 chunks ahead, then one more chunk
    # after each chunk's rows are consumed (preserves WAR order on ring reuse)
    for c in range(min(2, NCHUNK)):
        phase1_chunk(c)

    if dbg is not None:
        nc.sync.dma_start(dbg[0], e_ring[int(os.environ.get('KDG', '0'))][:])

    # ---- Phase 2: fused-scan DP -------------------------------------------
    v = nc.vector
    sr = [s0, s1]
    DPROWS = int(os.environ.get('KROWS', str(N)))
    for r in range(DPROWS):
        if r > 0 and r % 128 == 0 and r // 128 + 1 < NCHUNK:
            phase1_chunk(r // 128 + 1)
        prev = sr[r % 2]
        cur = sr[(r + 1) % 2]
        pap = prev[:]
        in0 = bass.AP(pap.tensor, pap.offset + 3,
                      [list(pap.ap[0]), [2, NC_], [-2, 2]])
        o_ap = cur[:].rearrange("p (c u) -> p c u", u=2)[:, 0:NC_, :]
        in1 = e_ring[(r // GROWS) % RING][:].rearrange(
            "p (r c u) -> p r c u", c=RL // 2, u=2)[:, r % GROWS][:, 0:NC_, :]
        _raw_scan(v, o_ap, in0, in1, 0.0)
        if r % RS == RS - 1:
            ev = r // RS
            v.reciprocal(m_buf[:, ev:ev + 1], cur[:, 101:102])
            v.tensor_scalar(cur[:, 0:2 * NC_], cur[:, 0:2 * NC_],
                            m_buf[:, ev:ev + 1], CAP, ALU.mult, ALU.min)

    if dbg is not None:
        sd = dbg[1].rearrange("p (a b) -> p a b", b=RL)
        nc.scalar.dma_start(sd[:, 0, :], s0[:])
        nc.scalar.dma_start(sd[:, 1, :], s1[:])
        nc.scalar.dma_start(dbg[2], m_buf[:])

    # ---- Phase 3: readout --------------------------------------------------
    # ACT's Ln table only covers inputs in ~(2^-64, 2^64); 1/m can reach e^59
    # at RS=16. Pre-scale by 2^-32: the ln(2^-32) offset is constant per event
    # and cancels in the loss combination (cmat columns sum to zero).
    ln_m = const.tile([NPROB, NEVT], F32)
    tsum = const.tile([NPROB, 1], F32)
    tfin = const.tile([NPROB, 1], F32)
    ln_s = const.tile([NPROB, 1], F32)
    nc.vector.tensor_scalar_mul(m_buf[:], m_buf[:], float(2.0 ** -32))
    nc.scalar.activation(ln_m[:], m_buf[:], ACTF.Ln)
    nc.vector.reduce_sum(tsum[:], ln_m[:], axis=mybir.AxisListType.X)
    s_last = sr[DPROWS % 2]
    nc.scalar.activation(ln_s[:], s_last[:, 101:102], ACTF.Ln)
    nc.vector.tensor_sub(tfin[:], ln_s[:], tsum[:])
    if dbg is not None:
        rd = dbg[3]
        nc.sync.dma_start(rd[:, 0:NEVT], ln_m[:])
        nc.sync.dma_start(rd[:, NEVT:NEVT + 1], tsum[:])
        nc.sync.dma_start(rd[:, NEVT + 1:NEVT + 2], ln_s[:])
        nc.sync.dma_start(rd[:, NEVT + 2:NEVT + 3], tfin[:])
    pf = ps_misc.tile([BPC, 1], F32)
    nc.tensor.matmul(pf[:], cm[:], tfin[:])
    ob = const.tile([BPC, 1], F32)
    nc.scalar.copy(ob[:], pf[:])
    nc.sync.dma_start(out, ob[:])


_NC_CACHE = None


def _get_nc():
    global _NC_CACHE
    if _NC_CACHE is None:
        _NC_CACHE = _build_nc()
    return _NC_CACHE


def _cmat_np():
    cmv = np.zeros((NPROB, BPC), np.float32)
    for m in range(BPC):
        cmv[m, m] = -2.0       # -gamma * T_xy
        cmv[4 + m, m] = 1.0    # +gamma/2 * T_xx
        cmv[8 + m, m] = 1.0    # +gamma/2 * T_yy
    return cmv


def _host_prep(xs, ys):
    """Build per-core device operands from [BPC, N, DIM] f32 slices."""
    seqs = [xs[b] for b in range(BPC)] + [ys[b] for b in range(BPC)]  # 8 x [N, D]
    a = np.empty((DIM, 8 * N), np.float32)
    b = np.zeros((DIM, 8 * BCOLS), np.float32)
    bn = np.full((1, 8 * BCOLS), NEG, np.float32)
    ab = np.empty((128, NCHUNK * 8), np.float32)
    for s, q in enumerate(seqs):
        qt = q.T  # [D, N]
        a[:, s * N:(s + 1) * N] = qt
        b[:, s * BCOLS + BW:s * BCOLS + BW + N] = qt
        nrm = -0.5 * np.sum(q * q, axis=1)  # [N]
        bn[0, s * BCOLS + BW:s * BCOLS + BW + N] = nrm
        for c in range(NCHUNK):
            ab[:, c * 8 + s] = nrm[c * 128:(c + 1) * 128]
    si = np.zeros((NPROB, RL), np.float32)
    si[:, 101] = 1.0
    ep = np.zeros((NPROB, GROWS * RL), np.float32)
    ep[:, :].reshape(NPROB, GROWS, RL)[:, :, 0::2] = 1.0
    return {
        "a_in": a.astype(np_bf16),
        "b_in": b.astype(np_bf16),
        "bn_in": bn.astype(np_bf16),
        "ab_in": ab,
        "si_in": si.astype(np_bf16),
        "ep_in": ep.astype(np_bf16),
        "cmat": _cmat_np(),
    }


def _in_maps(x, y):
    maps = []
    for k in range(NCORE):
        xs = x[BPC * k:BPC * (k + 1)]
        ys = y[BPC * k:BPC * (k + 1)]
        maps.append(_host_prep(xs, ys))
    return maps


def kernel(x: np.ndarray, y: np.ndarray) -> np.ndarray:
    x = np.ascontiguousarray(x, np.float32)
    y = np.ascontiguousarray(y, np.float32)
    B = x.shape[0]
    assert x.shape == (B, N, DIM) and B == BPC * NCORE
    nc = _get_nc()
    res = run_bass_kernel_spmd(nc, _in_maps(x, y), list(range(NCORE)))
    outs = [np.asarray(res.results[k]["out"]).reshape(BPC) for k in range(NCORE)]
    return np.concatenate(outs).astype(np.float32)


if __name__ == "__main__":
    xx = np.random.randn(32, N, DIM).astype(np.float32)
    yy = np.random.randn(32, N, DIM).astype(np.float32)
    print(kernel(xx, yy)[:4])
